# revision 1
# baseline (speedup 1.0000x reference)
"""ContextualConv2d Trainium2 kernel.

out = conv2d(x, weight, pad=1) + (c @ c_weight.T)[:, :, None, None] + bias[None, :, None, None]

Full shapes: x (32,128,64,64) f32, c (32,64), weight (256,128,3,3),
c_weight (256,64), bias (256,) -> out (32,256,64,64).

Strategy: data-parallel over batch across 8 NeuronCores (4 images each).
Per core the conv is an implicit GEMM: each image lives in SBUF with
stride-65 rows (a host-baked zero guard column after each 64-pixel row,
plus two zero rows for the H halo), so the +-1-column filter taps read
straight through zero guards and every tap is a uniform N=512 matmul
with inner-contiguous rhs. For each 128-wide C_out tile and each
512-column output block (8 image rows x 64 cols), 9 matmuls (one per
filter tap) accumulate into a PSUM bank using float32r operands (full
PE rate at N>=256, ~1.5e-4 rel err). The context bias
(c @ c_weight.T + bias) comes from one small on-device matmul per C_out
tile (a ones-row on the rhs folds in the channel bias) and is fused
into the PSUM->SBUF epilogue on ACT (co-tile 0) / DVE (co-tile 1).

Schedule: ~24 bf16 warmup matmuls keep the PE busy (HAM un-throttle)
while inputs stream; weights + images ride the scalar HWDGE ring,
context/outputs the sync ring; images 1-3 are prefetched one compute
pass ahead; output planes are stored in 4 x 512KB contiguous pieces so
the final piece doesn't sit whole on the kernel tail. Measured:
~160us HW exec, vs ~123us PE-matmul roofline for fp32r.
"""

import sys
import time
import types

import numpy as np

import concourse.tile as tile
from concourse import bacc, bass_utils, mybir


def _ensure_axon_hooks_shim():
    """concourse imports antenv.axon_hooks when BASS_TRACE is set; the agent
    image's antenv lacks it. Provide a null shim so tracing degrades to a
    warning instead of an ImportError."""
    try:
        import antenv

        if not hasattr(antenv, "axon_hooks"):
            try:
                from antenv import axon_hooks  # noqa: F401
            except ImportError:
                mod = types.ModuleType("antenv.axon_hooks")
                _state = {"hook": None}
                mod.set_axon_ntff_profile_hook = lambda h: _state.__setitem__(
                    "hook", h
                )
                mod.get_axon_ntff_profile_hook = lambda: _state["hook"]
                sys.modules["antenv.axon_hooks"] = mod
                antenv.axon_hooks = mod
    except Exception:
        pass


_ensure_axon_hooks_shim()

N_CORES = 8
N_FULL = 32
IMG = N_FULL // N_CORES  # images per core
CIN = 128
COUT = 256
H = W = 64
HW = H * W
KDIM = 3
CDIM = 64
XROWS = H + 2  # 2 zero rows for the H halo
CO_TILES = COUT // 128
ROWS_PER_BLK = 8
NBLK = H // ROWS_PER_BLK
BLK_N = ROWS_PER_BLK * W  # 512 = one fp32 PSUM bank
F32 = mybir.dt.float32
F32R = mybir.dt.float32r

_cached_nc = None


def _build():
    nc = bacc.Bacc(
        "TRN2",
        target_bir_lowering=False,
        debug=False,
        enable_asserts=False,
        num_devices=N_CORES,
    )
    x_d = nc.dram_tensor("x", (IMG, CIN, H, W + 1), F32R, kind="ExternalInput").ap()
    wt_d = nc.dram_tensor("wt", (KDIM * KDIM, CIN, COUT), F32R, kind="ExternalInput").ap()
    cb_d = nc.dram_tensor("cb", (CDIM + 1, IMG), F32R, kind="ExternalInput").ap()
    cwb_d = nc.dram_tensor("cwb", (CDIM + 1, COUT), F32R, kind="ExternalInput").ap()
    z_d = nc.dram_tensor("z", (CIN, W + 2), F32R, kind="ExternalInput").ap()
    out_d = nc.dram_tensor("out", (IMG, COUT, H, W), F32, kind="ExternalOutput").ap()

    with tile.TileContext(nc) as tc:
        with (
            tc.tile_pool(name="consts", bufs=1) as consts,
            tc.tile_pool(name="xbuf", bufs=1) as xbuf,
            tc.tile_pool(name="obuf", bufs=2) as obuf,
            tc.tile_pool(name="ps", bufs=5, space="PSUM") as pspool,
            tc.tile_pool(name="cps", bufs=1, space="PSUM") as cpspool,
            tc.tile_pool(name="wps", bufs=1, space="PSUM") as wpspool,
        ):
            # PE warmup: the PE idles for the first ~12us waiting on input
            # DMAs, and the HAM clock gate needs ~3.4us of sustained matmul
            # activity to lift the 1.2GHz cold throttle. Run dummy matmuls
            # on a small scratch tile so the real matmuls start at 2.4GHz.
            # The warmup matmuls only exist to keep the PE busy (HAM
            # un-throttle) while the real input DMAs land; their PSUM bank is
            # never read. bf16 zeros: memset is legal for bf16 and the PE
            # rate is the same.
            warm_sb = consts.tile([CIN, BLK_N], mybir.dt.bfloat16)
            nc.gpsimd.memset(warm_sb[:], 0.0)
            wps = wpspool.tile([128, BLK_N], F32)
            for _ in range(24):
                nc.tensor.matmul(
                    wps[:],
                    lhsT=warm_sb[:, 0:128],
                    rhs=warm_sb[:],
                    start=True,
                    stop=True,
                )

            # conv weights lead the scalar-ring FIFO (images follow); the
            # small context tensors and the output stores use the sync ring
            cwb_sb = consts.tile([CDIM + 1, COUT], F32R)
            nc.sync.dma_start(out=cwb_sb[:], in_=cwb_d)
            cb_sb = consts.tile([CDIM + 1, IMG], F32R)
            nc.sync.dma_start(out=cb_sb[:], in_=cb_d)
            w_sb = consts.tile([CIN, KDIM * KDIM * COUT], F32R)
            nc.scalar.dma_start(
                out=w_sb[:].rearrange("p (k o) -> p k o", o=COUT),
                in_=wt_d.transpose([1, 0, 2]),
            )

            # ctxb[t][co, n] = sum_d c_weight[co, d] * c[n, d] + bias[co]
            ctxb = []
            for t in range(CO_TILES):
                cps = cpspool.tile([128, IMG], F32, tag=f"cps{t}")
                nc.tensor.matmul(
                    cps[:],
                    lhsT=cwb_sb[:, t * 128 : (t + 1) * 128],
                    rhs=cb_sb[:],
                    start=True,
                    stop=True,
                )
                csb = consts.tile([128, IMG], F32, tag=f"ctxb{t}")
                nc.vector.tensor_copy(csb[:], cps[:])
                ctxb.append(csb)

            # per-image input planes with stride-65 rows: position
            # 1 + u*PWS + c holds image pixel (u-1, c); column PWS-1 of each
            # row is a zero guard (baked into the host-padded x tensor), and
            # rows 0 / XROWS-1 plus the leading element are zeroed from z_d.
            # The +-1-column taps then read straight through the guards
            # (which contribute zero), so every tap is a uniform N=512
            # matmul with inner-contiguous rhs and a plain 2D PSUM out.
            PWS = W + 1

            def load_image(n):
                """Emit the image-n load: top zero row + leading guard, two
                interior halves, bottom zero row. Fully contiguous DMAs."""
                # one extra row of slack: tap AP slices extend past the last
                # guard before the [:, :, :W] crop trims them
                # image 0 rides the sync ring in 16-row pieces so early conv
                # blocks start as soon as their rows land; later images are
                # prefetched on the scalar ring behind the weights with
                # plenty of slack
                xp = xbuf.tile([CIN, 1 + (XROWS + 1) * PWS], F32R, tag=f"ximg{n}")
                xflat = x_d[n].rearrange("p h w -> p (h w)")
                cut = 16 * PWS
                nc.scalar.dma_start(out=xp[:, 0 : 1 + PWS], in_=z_d[:, 0 : 1 + PWS])
                nc.scalar.dma_start(
                    out=xp[:, 1 + PWS : 1 + PWS + cut], in_=xflat[:, 0:cut]
                )
                nc.scalar.dma_start(
                    out=xp[:, 1 + PWS + cut : 1 + (XROWS - 1) * PWS],
                    in_=xflat[:, cut:],
                )
                nc.scalar.dma_start(
                    out=xp[:, 1 + (XROWS - 1) * PWS : 1 + XROWS * PWS],
                    in_=z_d[:, 0:PWS],
                )
                return xp

            xflats = {0: load_image(0)}

            for n in range(IMG):
                xf = xflats[n]
                for t in range(CO_TILES):
                    obig = obuf.tile([128, HW], F32)
                    for b in range(NBLK):
                        ps = pspool.tile([128, BLK_N], F32)
                        r0 = b * ROWS_PER_BLK
                        for i in range(KDIM * KDIM):
                            kh, kw = divmod(i, KDIM)
                            w0 = i * COUT + t * 128
                            o = 1 + (r0 + kh) * PWS + (kw - 1)
                            rhs = xf[:, o : o + ROWS_PER_BLK * PWS].rearrange(
                                "p (r c) -> p r c", c=PWS
                            )[:, :, :W]
                            nc.tensor.matmul(
                                ps[:],
                                lhsT=w_sb[:, w0 : w0 + 128],
                                rhs=rhs,
                                start=(i == 0),
                                stop=(i == KDIM * KDIM - 1),
                            )
                        oslice = obig[:, b * BLK_N : (b + 1) * BLK_N]
                        if t == 0:
                            nc.scalar.activation(
                                oslice,
                                ps[:],
                                mybir.ActivationFunctionType.Identity,
                                bias=ctxb[t][:, n : n + 1],
                                scale=1.0,
                            )
                        else:
                            nc.vector.tensor_scalar_add(
                                oslice, ps[:], ctxb[t][:, n : n + 1]
                            )
                    # split the 2MB plane store so the last piece doesn't sit
                    # whole on the kernel's critical tail
                    oflat = out_d[n, t * 128 : (t + 1) * 128].rearrange(
                        "o h w -> o (h w)"
                    )
                    for q in range(4):
                        nc.sync.dma_start(
                            out=oflat[:, q * (HW // 4) : (q + 1) * (HW // 4)],
                            in_=obig[:, q * (HW // 4) : (q + 1) * (HW // 4)],
                        )
                    # prefetch the next image while this one's second
                    # C_out tile computes
                    if t == 0 and n + 1 < IMG:
                        xflats[n + 1] = load_image(n + 1)
    nc.compile()
    return nc


def get_nc():
    global _cached_nc
    if _cached_nc is None:
        _cached_nc = _build()
    return _cached_nc


def prep_in_maps(x, c, weight, c_weight, bias):
    x = np.ascontiguousarray(np.asarray(x, dtype=np.float32))
    c = np.asarray(c, dtype=np.float32)
    weight = np.asarray(weight, dtype=np.float32)
    c_weight = np.asarray(c_weight, dtype=np.float32)
    bias = np.asarray(bias, dtype=np.float32)

    wt = np.ascontiguousarray(
        weight.transpose(2, 3, 1, 0).reshape(KDIM * KDIM, CIN, COUT)
    )
    cwb = np.ascontiguousarray(np.concatenate([c_weight.T, bias[None, :]], axis=0))
    z = np.zeros((CIN, W + 2), np.float32)
    xpad = np.zeros((N_FULL, CIN, H, W + 1), np.float32)
    xpad[:, :, :, :W] = x
    in_maps = []
    for i in range(N_CORES):
        xs = np.ascontiguousarray(xpad[i * IMG : (i + 1) * IMG])
        cb = np.ascontiguousarray(
            np.concatenate(
                [c[i * IMG : (i + 1) * IMG].T, np.ones((1, IMG), np.float32)], axis=0
            )
        )
        in_maps.append({"x": xs, "wt": wt, "cb": cb, "cwb": cwb, "z": z})
    return in_maps


def run(x, c, weight, c_weight, bias, trace=False):
    nc = get_nc()
    in_maps = prep_in_maps(x, c, weight, c_weight, bias)
    last_err = None
    for attempt in range(3):
        try:
            res = bass_utils.run_bass_kernel_spmd(
                nc, in_maps, core_ids=list(range(N_CORES)), trace=trace
            )
            break
        except Exception as e:  # noqa: BLE001
            # NRT_EXEC_UNIT_UNRECOVERABLE occasionally fires spuriously;
            # a reloaded execution recovers
            last_err = e
            time.sleep(2.0)
    else:
        raise last_err
    out = np.concatenate([res.results[i]["out"] for i in range(N_CORES)], axis=0)
    return out, res


def kernel(x, c, weight, c_weight, bias):
    out, _ = run(x, c, weight, c_weight, bias)
    return out



# revision 2
# speedup vs baseline: 1.0696x; 1.0696x over previous
"""ContextualConv2d Trainium2 kernel.

out = conv2d(x, weight, pad=1) + (c @ c_weight.T)[:, :, None, None] + bias[None, :, None, None]

Full shapes: x (32,128,64,64) f32, c (32,64), weight (256,128,3,3),
c_weight (256,64), bias (256,) -> out (32,256,64,64).

Strategy: data-parallel over batch across 8 NeuronCores (4 images each).
Per core the conv is an implicit GEMM: each image lives in SBUF with
stride-65 rows (a host-baked zero guard column after each 64-pixel row,
plus gpsimd-memset zero rows for the H halo), so the +-1-column filter
taps read straight through zero guards and every tap is a uniform N=512
matmul with inner-contiguous rhs. For each 128-wide C_out tile and each
512-column output block (8 image rows x 64 cols), 9 matmuls (one per
filter tap) accumulate into a PSUM bank.

Operands are bf16 (host-cast): fp32r matmuls run duty-throttled at
~236ns per 512-row matmul (avg util limit ~0.89 in the HAM counters)
while bf16 paces at ~216ns, and bf16 halves the input DMA bytes and
LDWEIGHTS time. PSUM accumulation stays fp32; measured rel l2 err
~1.5e-3 vs the 2e-2 gate. The context bias (c @ c_weight.T + bias,
fp32r) comes from one small on-device matmul per C_out tile (a ones-row
on the rhs folds in the channel bias) and is fused into the PSUM->SBUF
epilogue on ACT (co-tile 0) / DVE (co-tile 1).

Schedule: a few bf16 warmup matmuls keep the PE busy (HAM un-throttle)
while the first inputs land; weights (split per C_out tile, contiguous)
and images 1-3 ride the scalar HWDGE ring, the context tensors +
image 0 + output stores the sync ring; each 512-col output block is
stored right after its epilogue so the kernel tail only carries the
last block, whose epilogue and store are split across ACT+DVE and both
rings.
"""

import sys
import time
import types

import numpy as np
from ml_dtypes import bfloat16

import concourse.tile as tile
from concourse import bacc, bass_utils, mybir


def _ensure_axon_hooks_shim():
    """concourse imports antenv.axon_hooks when BASS_TRACE is set; the agent
    image's antenv lacks it. Provide a null shim so tracing degrades to a
    warning instead of an ImportError."""
    try:
        import antenv

        if not hasattr(antenv, "axon_hooks"):
            try:
                from antenv import axon_hooks  # noqa: F401
            except ImportError:
                mod = types.ModuleType("antenv.axon_hooks")
                _state = {"hook": None}
                mod.set_axon_ntff_profile_hook = lambda h: _state.__setitem__(
                    "hook", h
                )
                mod.get_axon_ntff_profile_hook = lambda: _state["hook"]
                sys.modules["antenv.axon_hooks"] = mod
                antenv.axon_hooks = mod
    except Exception:
        pass


_ensure_axon_hooks_shim()

N_CORES = 8
N_FULL = 32
IMG = N_FULL // N_CORES  # images per core
CIN = 128
COUT = 256
H = W = 64
HW = H * W
KDIM = 3
CDIM = 64
XROWS = H + 2  # 2 zero rows for the H halo
CO_TILES = COUT // 128
ROWS_PER_BLK = 8
NBLK = H // ROWS_PER_BLK
BLK_N = ROWS_PER_BLK * W  # 512 = one fp32 PSUM bank
N_WARM = 5
F32 = mybir.dt.float32
F32R = mybir.dt.float32r
BF16 = mybir.dt.bfloat16

_cached_nc = None


def _build():
    nc = bacc.Bacc(
        "TRN2",
        target_bir_lowering=False,
        debug=False,
        enable_asserts=False,
        num_devices=N_CORES,
    )
    x_d = nc.dram_tensor("x", (IMG, CIN, H, W + 1), BF16, kind="ExternalInput").ap()
    wt_d = nc.dram_tensor(
        "wt", (CO_TILES, CIN, KDIM * KDIM * 128), BF16, kind="ExternalInput"
    ).ap()
    cb_d = nc.dram_tensor("cb", (CDIM + 1, IMG), F32R, kind="ExternalInput").ap()
    cwb_d = nc.dram_tensor("cwb", (CDIM + 1, COUT), F32R, kind="ExternalInput").ap()
    out_d = nc.dram_tensor("out", (IMG, COUT, H, W), F32, kind="ExternalOutput").ap()

    with tile.TileContext(nc) as tc:
        with (
            tc.tile_pool(name="consts", bufs=1) as consts,
            tc.tile_pool(name="xbuf", bufs=1) as xbuf,
            tc.tile_pool(name="obuf", bufs=2) as obuf,
            tc.tile_pool(name="ps", bufs=5, space="PSUM") as pspool,
            tc.tile_pool(name="cps", bufs=1, space="PSUM") as cpspool,
            tc.tile_pool(name="wps", bufs=1, space="PSUM") as wpspool,
        ):
            # PE warmup: the HAM clock gate needs a few us of sustained matmul
            # activity to lift the cold throttle, and the real inputs take
            # ~10us (preamble + DMA) to land. A handful of dummy bf16 matmuls
            # on a memset scratch tile keeps the PE busy meanwhile; their
            # PSUM bank is never read.
            warm_sb = consts.tile([CIN, BLK_N], BF16)
            nc.gpsimd.memset(warm_sb[:], 0.0)
            wps = wpspool.tile([128, BLK_N], F32)
            for _ in range(N_WARM):
                nc.tensor.matmul(
                    wps[:],
                    lhsT=warm_sb[:, 0:128],
                    rhs=warm_sb[:],
                    start=True,
                    stop=True,
                )

            # sync ring: small context tensors first (they gate the ctx
            # matmuls right after warmup), then image 0; conv weights lead
            # the scalar ring, co-tile 0's weights alone gate the first conv
            # matmul. Images 1-3 follow on the scalar ring.
            cb_sb = consts.tile([CDIM + 1, IMG], F32R)
            nc.sync.dma_start(out=cb_sb[:], in_=cb_d)
            cwb_sb = consts.tile([CDIM + 1, COUT], F32R)
            nc.sync.dma_start(out=cwb_sb[:], in_=cwb_d)
            w_sb = []
            for t in range(CO_TILES):
                wt_sb = consts.tile([CIN, KDIM * KDIM * 128], BF16, tag=f"w{t}")
                nc.scalar.dma_start(out=wt_sb[:], in_=wt_d[t])
                w_sb.append(wt_sb)

            # per-image input planes with stride-65 rows: position
            # 1 + u*PWS + c holds image pixel (u-1, c); column PWS-1 of each
            # row is a zero guard (baked into the host-padded x tensor), and
            # rows 0 / XROWS-1 plus the leading element are memset to zero.
            # The +-1-column taps then read straight through the guards
            # (which contribute zero), so every tap is a uniform N=512
            # matmul with inner-contiguous rhs and a plain 2D PSUM out.
            PWS = W + 1

            def load_image(n, ring):
                """Emit the image-n load: gpsimd-memset halo rows, then the
                interior in three row pieces (16/24/24) so early conv blocks
                start as soon as their rows land. Fully contiguous DMAs."""
                # one extra row of slack: tap AP slices extend past the last
                # guard before the [:, :, :W] crop trims them
                xp = xbuf.tile([CIN, 1 + (XROWS + 1) * PWS], BF16, tag=f"ximg{n}")
                nc.gpsimd.memset(xp[:, 0 : 1 + PWS], 0.0)
                nc.gpsimd.memset(
                    xp[:, 1 + (XROWS - 1) * PWS : 1 + XROWS * PWS], 0.0
                )
                xflat = x_d[n].rearrange("p h w -> p (h w)")
                for r0, r1 in ((0, 16), (16, 40), (40, 64)):
                    ring.dma_start(
                        out=xp[:, 1 + PWS + r0 * PWS : 1 + PWS + r1 * PWS],
                        in_=xflat[:, r0 * PWS : r1 * PWS],
                    )
                return xp

            xflats = {0: load_image(0, nc.sync)}

            # ctxb[t][co, n] = sum_d c_weight[co, d] * c[n, d] + bias[co]
            ctxb = []
            for t in range(CO_TILES):
                cps = cpspool.tile([128, IMG], F32, tag=f"cps{t}")
                nc.tensor.matmul(
                    cps[:],
                    lhsT=cwb_sb[:, t * 128 : (t + 1) * 128],
                    rhs=cb_sb[:],
                    start=True,
                    stop=True,
                )
                csb = consts.tile([128, IMG], F32, tag=f"ctxb{t}")
                nc.vector.tensor_copy(csb[:], cps[:])
                ctxb.append(csb)

            for n in range(IMG):
                xf = xflats[n]
                for t in range(CO_TILES):
                    obig = obuf.tile([128, HW], F32)
                    oflat = out_d[n, t * 128 : (t + 1) * 128].rearrange(
                        "o h w -> o (h w)"
                    )
                    for b in range(NBLK):
                        ps = pspool.tile([128, BLK_N], F32)
                        r0 = b * ROWS_PER_BLK
                        for i in range(KDIM * KDIM):
                            kh, kw = divmod(i, KDIM)
                            o = 1 + (r0 + kh) * PWS + (kw - 1)
                            rhs = xf[:, o : o + ROWS_PER_BLK * PWS].rearrange(
                                "p (r c) -> p r c", c=PWS
                            )[:, :, :W]
                            nc.tensor.matmul(
                                ps[:],
                                lhsT=w_sb[t][:, i * 128 : (i + 1) * 128],
                                rhs=rhs,
                                start=(i == 0),
                                stop=(i == KDIM * KDIM - 1),
                            )
                        oslice = obig[:, b * BLK_N : (b + 1) * BLK_N]
                        last = n == IMG - 1 and t == CO_TILES - 1 and b == NBLK - 1
                        if last:
                            # split the final epilogue + store across ACT/DVE
                            # and both rings so the kernel tail is half a
                            # block, not a whole plane
                            hb = BLK_N // 2
                            c0 = b * BLK_N
                            nc.scalar.activation(
                                obig[:, c0 : c0 + hb],
                                ps[:, 0:hb],
                                mybir.ActivationFunctionType.Identity,
                                bias=ctxb[t][:, n : n + 1],
                                scale=1.0,
                            )
                            nc.vector.tensor_scalar_add(
                                obig[:, c0 + hb : c0 + BLK_N],
                                ps[:, hb:BLK_N],
                                ctxb[t][:, n : n + 1],
                            )
                            nc.sync.dma_start(
                                out=oflat[:, c0 : c0 + hb],
                                in_=obig[:, c0 : c0 + hb],
                            )
                            nc.scalar.dma_start(
                                out=oflat[:, c0 + hb : c0 + BLK_N],
                                in_=obig[:, c0 + hb : c0 + BLK_N],
                            )
                            continue
                        if t == 0:
                            nc.scalar.activation(
                                oslice,
                                ps[:],
                                mybir.ActivationFunctionType.Identity,
                                bias=ctxb[t][:, n : n + 1],
                                scale=1.0,
                            )
                        else:
                            nc.vector.tensor_scalar_add(
                                oslice, ps[:], ctxb[t][:, n : n + 1]
                            )
                        # store each block as soon as its epilogue lands so
                        # the plane never sits whole on the kernel tail
                        nc.sync.dma_start(
                            out=oflat[:, b * BLK_N : (b + 1) * BLK_N],
                            in_=oslice,
                        )
                    # prefetch the next image while this one's second
                    # C_out tile computes
                    if t == 0 and n + 1 < IMG:
                        xflats[n + 1] = load_image(n + 1, nc.scalar)
    nc.compile()
    return nc


def get_nc():
    global _cached_nc
    if _cached_nc is None:
        _cached_nc = _build()
    return _cached_nc


def prep_in_maps(x, c, weight, c_weight, bias):
    x = np.ascontiguousarray(np.asarray(x, dtype=np.float32))
    c = np.asarray(c, dtype=np.float32)
    weight = np.asarray(weight, dtype=np.float32)
    c_weight = np.asarray(c_weight, dtype=np.float32)
    bias = np.asarray(bias, dtype=np.float32)

    # (CO_TILES, CIN, KDIM*KDIM*128) bf16, contiguous per co-tile
    whwio = weight.transpose(1, 2, 3, 0).reshape(CIN, KDIM * KDIM, COUT)
    wt = np.ascontiguousarray(
        np.stack(
            [
                whwio[:, :, t * 128 : (t + 1) * 128].reshape(CIN, KDIM * KDIM * 128)
                for t in range(CO_TILES)
            ]
        )
    ).astype(bfloat16)
    cwb = np.ascontiguousarray(np.concatenate([c_weight.T, bias[None, :]], axis=0))
    xpad = np.zeros((N_FULL, CIN, H, W + 1), np.float32)
    xpad[:, :, :, :W] = x
    xpad = xpad.astype(bfloat16)
    in_maps = []
    for i in range(N_CORES):
        xs = np.ascontiguousarray(xpad[i * IMG : (i + 1) * IMG])
        cb = np.ascontiguousarray(
            np.concatenate(
                [c[i * IMG : (i + 1) * IMG].T, np.ones((1, IMG), np.float32)], axis=0
            )
        )
        in_maps.append({"x": xs, "wt": wt, "cb": cb, "cwb": cwb})
    return in_maps


def run(x, c, weight, c_weight, bias, trace=False):
    nc = get_nc()
    in_maps = prep_in_maps(x, c, weight, c_weight, bias)
    last_err = None
    for attempt in range(3):
        try:
            res = bass_utils.run_bass_kernel_spmd(
                nc, in_maps, core_ids=list(range(N_CORES)), trace=trace
            )
            break
        except Exception as e:  # noqa: BLE001
            # NRT_EXEC_UNIT_UNRECOVERABLE occasionally fires spuriously;
            # a reloaded execution recovers
            last_err = e
            time.sleep(2.0)
    else:
        raise last_err
    out = np.concatenate([res.results[i]["out"] for i in range(N_CORES)], axis=0)
    return out, res


def kernel(x, c, weight, c_weight, bias):
    out, _ = run(x, c, weight, c_weight, bias)
    return out


# revision 11
# speedup vs baseline: 1.1146x; 1.0421x over previous
"""ContextualConv2d Trainium2 kernel.

out = conv2d(x, weight, pad=1) + (c @ c_weight.T)[:, :, None, None] + bias[None, :, None, None]

Full shapes: x (32,128,64,64) f32, c (32,64), weight (256,128,3,3),
c_weight (256,64), bias (256,) -> out (32,256,64,64).

Strategy: data-parallel over batch across 8 NeuronCores (4 images each).
Per core the conv is an implicit GEMM: each image lives in SBUF with
stride-65 rows (a host-baked zero guard column after each 64-pixel row,
plus gpsimd-memset zero rows for the H halo), so the +-1-column filter
taps read straight through zero guards and every tap is a uniform N=512
matmul with inner-contiguous rhs. For each 128-wide C_out tile and each
512-column output block (8 image rows x 64 cols), 9 matmuls (one per
filter tap) accumulate into a PSUM bank.

Operands are bf16 (host-cast): fp32r matmuls run duty-throttled at
~236ns per 512-row matmul (avg util limit ~0.89 in the HAM counters)
while bf16 paces at ~216ns, and bf16 halves the input DMA bytes and
LDWEIGHTS time. PSUM accumulation stays fp32; measured rel l2 err
~1.5e-3 vs the 2e-2 gate. The context bias (c @ c_weight.T + bias,
fp32r) comes from one small on-device matmul per C_out tile (a ones-row
on the rhs folds in the channel bias) and is fused into the PSUM->SBUF
epilogue on ACT (co-tile 0) / DVE (co-tile 1).

Schedule: a few bf16 warmup matmuls keep the PE busy (HAM un-throttle)
while the first inputs land; weights (split per C_out tile, contiguous)
and images 1-3 ride the scalar HWDGE ring, the context tensors +
image 0 + output stores the sync ring; each 512-col output block is
stored right after its epilogue so the kernel tail only carries the
last block, whose epilogue and store are split across ACT+DVE and both
rings.
"""

import sys
import time
import types

import numpy as np
from ml_dtypes import bfloat16

import concourse.tile as tile
from concourse import bacc, bass_utils, mybir


def _ensure_axon_hooks_shim():
    """concourse imports antenv.axon_hooks when BASS_TRACE is set; the agent
    image's antenv lacks it. Provide a null shim so tracing degrades to a
    warning instead of an ImportError."""
    try:
        import antenv

        if not hasattr(antenv, "axon_hooks"):
            try:
                from antenv import axon_hooks  # noqa: F401
            except ImportError:
                mod = types.ModuleType("antenv.axon_hooks")
                _state = {"hook": None}
                mod.set_axon_ntff_profile_hook = lambda h: _state.__setitem__(
                    "hook", h
                )
                mod.get_axon_ntff_profile_hook = lambda: _state["hook"]
                sys.modules["antenv.axon_hooks"] = mod
                antenv.axon_hooks = mod
    except Exception:
        pass


_ensure_axon_hooks_shim()

N_CORES = 8
N_FULL = 32
IMG = N_FULL // N_CORES  # images per core
CIN = 128
COUT = 256
H = W = 64
HW = H * W
KDIM = 3
CDIM = 64
XROWS = H + 2  # 2 zero rows for the H halo
CO_TILES = COUT // 128
ROWS_PER_BLK = 8
NBLK = H // ROWS_PER_BLK
BLK_N = ROWS_PER_BLK * W  # 512 = one fp32 PSUM bank
N_WARM = 6
F32 = mybir.dt.float32
F32R = mybir.dt.float32r
BF16 = mybir.dt.bfloat16

_cached_nc = None


def _build():
    nc = bacc.Bacc(
        "TRN2",
        target_bir_lowering=False,
        debug=False,
        enable_asserts=False,
        num_devices=N_CORES,
    )
    x_d = nc.dram_tensor("x", (IMG, CIN, H, W + 1), BF16, kind="ExternalInput").ap()
    wt_d = nc.dram_tensor(
        "wt", (CO_TILES, CIN, KDIM * KDIM * 128), BF16, kind="ExternalInput"
    ).ap()
    # c_weight.T, bias row, then c.T columns: one contiguous 1KB-per-partition
    # DMA (a separate (65,4) cb tensor degenerates to 16B packets)
    cwb_d = nc.dram_tensor(
        "cwb", (CDIM + 1, COUT + IMG), F32R, kind="ExternalInput"
    ).ap()
    out_d = nc.dram_tensor("out", (IMG, COUT, H, W), BF16, kind="ExternalOutput").ap()

    with tile.TileContext(nc) as tc:
        with (
            tc.tile_pool(name="consts", bufs=1) as consts,
            tc.tile_pool(name="xbuf", bufs=1) as xbuf,
            tc.tile_pool(name="obuf", bufs=2) as obuf,
            tc.tile_pool(name="ps", bufs=5, space="PSUM") as pspool,
            tc.tile_pool(name="cps", bufs=1, space="PSUM") as cpspool,
            tc.tile_pool(name="wps", bufs=1, space="PSUM") as wpspool,
        ):
            # PE warmup: the HAM clock gate needs a few us of sustained matmul
            # activity to lift the cold throttle, and the real inputs take
            # ~10us (preamble + DMA) to land. A handful of dummy bf16 matmuls
            # on a memset scratch tile keeps the PE busy meanwhile; their
            # PSUM bank is never read.
            warm_sb = consts.tile([CIN, BLK_N], BF16)
            nc.gpsimd.memset(warm_sb[:], 0.0)
            wps = wpspool.tile([128, BLK_N], F32)
            for _ in range(N_WARM):
                nc.tensor.matmul(
                    wps[:],
                    lhsT=warm_sb[:, 0:128],
                    rhs=warm_sb[:],
                    start=True,
                    stop=True,
                )

            # sync ring: only the small context tensor up front (it gates the
            # ctx matmuls right after warmup); everything the first conv
            # blocks need rides the fast scalar ring in need-order so the
            # critical path never shares queue bandwidth:
            #   w0[taps 0-2], x0[rows 0-10], w0[taps 3-8], x0[rows 10-32],
            #   x0[rows 32-64], w1, x1..x3
            cwb_sb = consts.tile([CDIM + 1, COUT + IMG], F32R)
            nc.sync.dma_start(out=cwb_sb[:], in_=cwb_d)
            w_sb = []
            for t in range(CO_TILES):
                wt_sb = consts.tile([CIN, KDIM * KDIM * 128], BF16, tag=f"w{t}")
                w_sb.append(wt_sb)

            # per-image input planes with stride-65 rows: position
            # 1 + u*PWS + c holds image pixel (u-1, c); column PWS-1 of each
            # row is a zero guard (baked into the host-padded x tensor), and
            # rows 0 / XROWS-1 plus the leading element are memset to zero.
            # The +-1-column taps then read straight through the guards
            # (which contribute zero), so every tap is a uniform N=512
            # matmul with inner-contiguous rhs and a plain 2D PSUM out.
            PWS = W + 1

            def alloc_image(n):
                # one extra row of slack: tap AP slices extend past the last
                # guard before the [:, :, :W] crop trims them
                xp = xbuf.tile([CIN, 1 + (XROWS + 1) * PWS], BF16, tag=f"ximg{n}")
                nc.gpsimd.memset(xp[:, 0 : 1 + PWS], 0.0)
                nc.gpsimd.memset(
                    xp[:, 1 + (XROWS - 1) * PWS : 1 + XROWS * PWS], 0.0
                )
                return xp

            def load_rows(xp, n, r0, r1):
                xflat = x_d[n].rearrange("p h w -> p (h w)")
                nc.scalar.dma_start(
                    out=xp[:, 1 + PWS + r0 * PWS : 1 + PWS + r1 * PWS],
                    in_=xflat[:, r0 * PWS : r1 * PWS],
                )

            def load_image(n):
                """gpsimd-memset halo rows, interior in three row pieces so
                early conv blocks start as soon as their rows land."""
                xp = alloc_image(n)
                for r0, r1 in ((0, 10), (10, 32), (32, 64)):
                    load_rows(xp, n, r0, r1)
                return xp

            # critical-path interleave on the scalar ring (see above)
            xp0 = alloc_image(0)
            nc.scalar.dma_start(out=w_sb[0][:, 0 : 3 * 128], in_=wt_d[0, :, 0 : 3 * 128])
            load_rows(xp0, 0, 0, 10)
            nc.scalar.dma_start(
                out=w_sb[0][:, 3 * 128 :], in_=wt_d[0, :, 3 * 128 :]
            )
            load_rows(xp0, 0, 10, 32)
            load_rows(xp0, 0, 32, 64)
            nc.scalar.dma_start(out=w_sb[1][:], in_=wt_d[1])
            xflats = {0: xp0}

            # ctxb[t][co, n] = sum_d c_weight[co, d] * c[n, d] + bias[co]
            ctxb = []
            for t in range(CO_TILES):
                cps = cpspool.tile([128, IMG], F32, tag=f"cps{t}")
                nc.tensor.matmul(
                    cps[:],
                    lhsT=cwb_sb[:, t * 128 : (t + 1) * 128],
                    rhs=cwb_sb[:, COUT : COUT + IMG],
                    start=True,
                    stop=True,
                )
                csb = consts.tile([128, IMG], F32, tag=f"ctxb{t}")
                nc.vector.tensor_copy(csb[:], cps[:])
                ctxb.append(csb)

            for n in range(IMG):
                xf = xflats[n]
                for t in range(CO_TILES):
                    obig = obuf.tile([128, HW], BF16)
                    oflat = out_d[n, t * 128 : (t + 1) * 128].rearrange(
                        "o h w -> o (h w)"
                    )
                    for b in range(NBLK):
                        ps = pspool.tile([128, BLK_N], F32)
                        r0 = b * ROWS_PER_BLK
                        for i in range(KDIM * KDIM):
                            kh, kw = divmod(i, KDIM)
                            o = 1 + (r0 + kh) * PWS + (kw - 1)
                            rhs = xf[:, o : o + ROWS_PER_BLK * PWS].rearrange(
                                "p (r c) -> p r c", c=PWS
                            )[:, :, :W]
                            nc.tensor.matmul(
                                ps[:],
                                lhsT=w_sb[t][:, i * 128 : (i + 1) * 128],
                                rhs=rhs,
                                start=(i == 0),
                                stop=(i == KDIM * KDIM - 1),
                            )
                        oslice = obig[:, b * BLK_N : (b + 1) * BLK_N]
                        last = n == IMG - 1 and t == CO_TILES - 1 and b == NBLK - 1
                        if last:
                            # split the final epilogue + store across ACT/DVE
                            # and both rings so the kernel tail is half a
                            # block, not a whole plane
                            hb = BLK_N // 2
                            c0 = b * BLK_N
                            nc.scalar.activation(
                                obig[:, c0 : c0 + hb],
                                ps[:, 0:hb],
                                mybir.ActivationFunctionType.Identity,
                                bias=ctxb[t][:, n : n + 1],
                                scale=1.0,
                            )
                            nc.vector.tensor_scalar_add(
                                obig[:, c0 + hb : c0 + BLK_N],
                                ps[:, hb:BLK_N],
                                ctxb[t][:, n : n + 1],
                            )
                            nc.sync.dma_start(
                                out=oflat[:, c0 : c0 + hb],
                                in_=obig[:, c0 : c0 + hb],
                            )
                            nc.scalar.dma_start(
                                out=oflat[:, c0 + hb : c0 + BLK_N],
                                in_=obig[:, c0 + hb : c0 + BLK_N],
                            )
                            continue
                        if t == 0:
                            nc.scalar.activation(
                                oslice,
                                ps[:],
                                mybir.ActivationFunctionType.Identity,
                                bias=ctxb[t][:, n : n + 1],
                                scale=1.0,
                            )
                        else:
                            nc.vector.tensor_scalar_add(
                                oslice, ps[:], ctxb[t][:, n : n + 1]
                            )
                        # store each block as soon as its epilogue lands so
                        # the plane never sits whole on the kernel tail
                        nc.sync.dma_start(
                            out=oflat[:, b * BLK_N : (b + 1) * BLK_N],
                            in_=oslice,
                        )
                    # prefetch the next image while this one's second
                    # C_out tile computes
                    if t == 0 and n + 1 < IMG:
                        xflats[n + 1] = load_image(n + 1)
    nc.compile()
    return nc


def get_nc():
    global _cached_nc
    if _cached_nc is None:
        _cached_nc = _build()
    return _cached_nc


def prep_in_maps(x, c, weight, c_weight, bias):
    x = np.ascontiguousarray(np.asarray(x, dtype=np.float32))
    c = np.asarray(c, dtype=np.float32)
    weight = np.asarray(weight, dtype=np.float32)
    c_weight = np.asarray(c_weight, dtype=np.float32)
    bias = np.asarray(bias, dtype=np.float32)

    # (CO_TILES, CIN, KDIM*KDIM*128) bf16, contiguous per co-tile
    whwio = weight.transpose(1, 2, 3, 0).reshape(CIN, KDIM * KDIM, COUT)
    wt = np.ascontiguousarray(
        np.stack(
            [
                whwio[:, :, t * 128 : (t + 1) * 128].reshape(CIN, KDIM * KDIM * 128)
                for t in range(CO_TILES)
            ]
        )
    ).astype(bfloat16)
    cwb0 = np.concatenate([c_weight.T, bias[None, :]], axis=0)  # (65, 256)
    xpad = np.zeros((N_FULL, CIN, H, W + 1), np.float32)
    xpad[:, :, :, :W] = x
    xpad = xpad.astype(bfloat16)
    in_maps = []
    for i in range(N_CORES):
        xs = np.ascontiguousarray(xpad[i * IMG : (i + 1) * IMG])
        cb = np.concatenate(
            [c[i * IMG : (i + 1) * IMG].T, np.ones((1, IMG), np.float32)], axis=0
        )
        cwb = np.ascontiguousarray(np.concatenate([cwb0, cb], axis=1))
        in_maps.append({"x": xs, "wt": wt, "cwb": cwb})
    return in_maps


def run(x, c, weight, c_weight, bias, trace=False):
    nc = get_nc()
    in_maps = prep_in_maps(x, c, weight, c_weight, bias)
    last_err = None
    for attempt in range(3):
        try:
            res = bass_utils.run_bass_kernel_spmd(
                nc, in_maps, core_ids=list(range(N_CORES)), trace=trace
            )
            break
        except Exception as e:  # noqa: BLE001
            # NRT_EXEC_UNIT_UNRECOVERABLE occasionally fires spuriously;
            # a reloaded execution recovers
            last_err = e
            time.sleep(2.0)
    else:
        raise last_err
    out = np.concatenate(
        [np.asarray(res.results[i]["out"]).astype(np.float32) for i in range(N_CORES)],
        axis=0,
    )
    return out, res


def kernel(x, c, weight, c_weight, bias):
    out, _ = run(x, c, weight, c_weight, bias)
    return out
